# revision 15
# baseline (speedup 1.0000x reference)
"""GAT (graph attention) layer on 8 Trainium2 NeuronCores.

Strategy (edge sharding by destination node):
  - Host: append self-loops, map each dst node to a (device, tile, pos)
    slot (nodes dealt round-robin by degree rank so per-tile edge counts
    balance across devices), pack per-device edge arrays padded to a
    shared block structure so one SPMD program serves all 8 cores.
  - Phase 0 (replicated): h_ext = x @ [W | W@As] via PE with a host-
    pre-transposed x_T; rows [h(128)|a_src(4)|pad] stored as two DRAM
    tables (row id < 32768 / >= 32768) of 768B rows for int16 dma_gather.
    A second small pass computes a_dst for the device's own (permuted)
    nodes into a 256B-row table.
  - Phase 1: per 128-dst-node tile: dma_gather of h_ext rows by src
    (lo/hi subsets in separate block runs), dma_gather of a_dst rows by
    local dst id, p = exp(leaky_relu(a_src+a_dst)), msg = [p*h | p],
    one-hot S via batched is_equal, one PE matmul per 128-edge block
    accumulating weighted sums + softmax denominators into PSUM.  Each
    device owns its dst nodes completely -> no collectives.
  - Phase 2: alpha = p * (1/denom)[dst] via 256B-row dma_gather.
  - Host: un-permute rows, add bias, scatter alpha to original order.
"""

import os
import sys

import numpy as np

sys.path.insert(0, "/opt/trn_rl_repo")

N = 50000
E = 1600000
IN_CH = 128
OUT_CH = 32
HEADS = 4
NEG_SLOPE = 0.2

N_DEV = 8
P = 128
N_TILES = 392
NPAD = N_TILES * P       # 50176
TPD = N_TILES // N_DEV   # 49
NLOC = TPD * P           # 6272
GROUP = 64
LO = 32768               # first LO node ids in table-lo, rest in table-hi
NHI = NPAD - LO          # 17408
ROW = 192                # 768B gathered h_ext row (132 used)
AROW = 64                # 256B a_dst / invden row (4 used)
CHUNK = 8                # gather chunk in blocks (1024 idx, <=3072 desc)

_last_results = None
_last_exec_s = None


def _prepare(edge_index):
    src = np.concatenate([edge_index[0], np.arange(N, dtype=np.int64)]).astype(np.int64)
    dst = np.concatenate([edge_index[1], np.arange(N, dtype=np.int64)]).astype(np.int64)
    E2 = src.shape[0]

    deg = np.bincount(dst, minlength=N)
    order = np.argsort(-deg, kind="stable")
    rank = np.empty(N, dtype=np.int64)
    rank[order] = np.arange(N)
    tile_of = rank % N_TILES
    pos_of = rank // N_TILES
    dev_of = (tile_of % N_DEV).astype(np.int64)
    slot_of = tile_of // N_DEV

    ed_dev = dev_of[dst]
    ed_slot = slot_of[dst]
    ed_pos = pos_of[dst]
    ed_g = ed_pos // GROUP
    ed_sub = (src >= LO).astype(np.int64)

    # cell = (dev, slot, g, sub)
    cell = (((ed_dev * TPD + ed_slot) * 2 + ed_g) * 2 + ed_sub)
    n_cells = N_DEV * TPD * 4
    ordr = np.argsort(cell, kind="stable")
    cell_sorted = cell[ordr]
    counts = np.bincount(cell_sorted, minlength=n_cells).reshape(N_DEV, TPD, 2, 2)

    B4 = -(-counts.max(axis=0) // P)          # [TPD, 2, 2] blocks per (g, sub)
    empty = B4.sum(axis=(1, 2)) == 0
    B4[empty, 0, 0] = 1

    # within-tile block order: [g0s0, g1s0, g0s1, g1s1] (lo run then hi run)
    tile_kb = B4.sum(axis=(1, 2))             # [TPD]
    tile_c0 = np.concatenate([[0], np.cumsum(tile_kb)[:-1]])
    off4 = np.zeros((TPD, 2, 2), dtype=np.int64)
    off4[:, 0, 0] = 0
    off4[:, 1, 0] = B4[:, 0, 0]
    off4[:, 0, 1] = B4[:, 0, 0] + B4[:, 1, 0]
    off4[:, 1, 1] = off4[:, 0, 1] + B4[:, 0, 1]
    B_dev = int(tile_kb.sum())

    cell_start = np.zeros(n_cells + 1, dtype=np.int64)
    np.cumsum(np.bincount(cell_sorted, minlength=n_cells), out=cell_start[1:])
    k = np.arange(E2, dtype=np.int64) - cell_start[cell_sorted]

    e_dev = ed_dev[ordr]
    e_slot = ed_slot[ordr]
    e_g = ed_g[ordr]
    e_sub = ed_sub[ordr]
    c_idx = tile_c0[e_slot] + off4[e_slot, e_g, e_sub] + k // P
    p_idx = k % P

    e_src16 = (src[ordr] - e_sub * LO).astype(np.int16)
    e_dstl = (e_slot * P + ed_pos[ordr]).astype(np.int16)

    # universal wrapped-16 idx layout: [p%16, c*8 + p//16]
    idxT1 = np.zeros((N_DEV, 16, B_dev * 8), dtype=np.int16)
    idxT2 = np.zeros((N_DEV, 16, B_dev * 8), dtype=np.int16)
    col = c_idx * 8 + p_idx // 16
    row = p_idx % 16
    idxT1[e_dev, row, col] = e_src16
    idxT2[e_dev, row, col] = e_dstl

    dl64_arr = np.full((N_DEV, P, B_dev), -1.0, dtype=np.float32)
    orig_arr = np.full((N_DEV, P, B_dev), -1, dtype=np.int64)
    dl64_arr[e_dev, p_idx, c_idx] = (ed_pos[ordr] % GROUP).astype(np.float32)
    orig_arr[e_dev, p_idx, c_idx] = ordr

    node_map = np.full((N_DEV, NLOC), -1, dtype=np.int64)
    node_map[dev_of, slot_of * P + pos_of] = np.arange(N)

    return dict(
        B4=B4, tile_kb=tile_kb, B_dev=B_dev,
        idxT1=np.tile(idxT1, (1, 8, 1)), idxT2=np.tile(idxT2, (1, 8, 1)),
        dl64=dl64_arr, orig=orig_arr, node_map=node_map, E2=E2,
    )


_program_cache = {}


def _build_program(B4, B_dev):
    import concourse.bacc as bacc
    import concourse.bass as bass
    import concourse.mybir as mybir
    import concourse.tile as tile
    from concourse import library_config
    from contextlib import ExitStack

    f32 = mybir.dt.float32
    i16 = mybir.dt.int16
    AF = mybir.ActivationFunctionType
    OP = mybir.AluOpType

    nc = bacc.Bacc("TRN2", target_bir_lowering=False, debug=False)

    xT = nc.dram_tensor("xT", [P, NPAD], f32, kind="ExternalInput")
    xTp = nc.dram_tensor("xTp", [P, NLOC], f32, kind="ExternalInput")
    Wf = nc.dram_tensor("Wf", [P, ROW], f32, kind="ExternalInput")
    Wd = nc.dram_tensor("Wd", [P, AROW], f32, kind="ExternalInput")
    iota = nc.dram_tensor("iota64", [P, GROUP], f32, kind="ExternalInput")
    idxT1 = nc.dram_tensor("idxT1", [P, B_dev * 8], i16, kind="ExternalInput")
    idxT2 = nc.dram_tensor("idxT2", [P, B_dev * 8], i16, kind="ExternalInput")
    dl64I = nc.dram_tensor("dl64I", [P, B_dev], f32, kind="ExternalInput")

    out_d = nc.dram_tensor("out", [NLOC, IN_CH], f32, kind="ExternalOutput")
    alpha_d = nc.dram_tensor("alpha", [P, B_dev * 4], f32, kind="ExternalOutput")

    t1lo = nc.dram_tensor("t1lo", [LO, ROW], f32)
    t1hi = nc.dram_tensor("t1hi", [NHI, ROW], f32)
    t2 = nc.dram_tensor("t2", [NLOC, AROW], f32)
    invd = nc.dram_tensor("invd", [NLOC, AROW], f32)
    pdram = nc.dram_tensor("pdram", [P, B_dev * 4], f32)

    kb_list = B4.sum(axis=(1, 2)).astype(int)
    kblo_list = B4[:, :, 0].sum(axis=1).astype(int)

    with tile.TileContext(nc) as tc, ExitStack() as ctx:
        consts = ctx.enter_context(tc.tile_pool(name="consts", bufs=1))
        nc.gpsimd.load_library(library_config.mlp)
        W_sb = consts.tile([P, ROW], f32)
        nc.sync.dma_start(W_sb[:], Wf[:, :])
        Wd_sb = consts.tile([P, AROW], f32)
        nc.sync.dma_start(Wd_sb[:], Wd[:, :])
        iota_sb = consts.tile([P, GROUP], f32)
        nc.sync.dma_start(iota_sb[:], iota[:, :])

        # ---------------- Phase 0: h_ext tables + a_dst table ----------------
        with ExitStack() as ctx0:
            xp = ctx0.enter_context(tc.tile_pool(name="xp", bufs=3))
            h0 = ctx0.enter_context(tc.tile_pool(name="h0", bufs=4))
            ps0 = ctx0.enter_context(tc.tile_pool(name="ps0", bufs=4, space="PSUM"))
            for chb in range(NPAD // 512):
                xt = xp.tile([P, 512], f32, tag="xt")
                nc.sync.dma_start(xt[:], xT[:, chb * 512:(chb + 1) * 512])
                for j in range(4):
                    ti = chb * 4 + j
                    ps = ps0.tile([P, ROW], f32, tag="ps")
                    nc.tensor.matmul(
                        out=ps[:], lhsT=xt[:, j * P:(j + 1) * P], rhs=W_sb[:],
                        start=True, stop=True,
                    )
                    hsb = h0.tile([P, ROW], f32, tag="hsb")
                    if j % 2 == 0:
                        nc.vector.tensor_copy(out=hsb[:], in_=ps[:])
                    else:
                        nc.scalar.copy(out=hsb[:], in_=ps[:])
                    # full 768B rows -> one contiguous 98KB store per tile
                    if ti < LO // P:
                        dest = t1lo[ti * P:(ti + 1) * P, :]
                    else:
                        r0 = ti * P - LO
                        dest = t1hi[r0:r0 + P, :]
                    nc.sync.dma_start(dest, hsb[:])
            # a_dst for this device's own nodes, in local (permuted) order
            for t in range(TPD):
                xt = xp.tile([P, P], f32, tag="xt2")
                nc.sync.dma_start(xt[:], xTp[:, t * P:(t + 1) * P])
                ps = ps0.tile([P, AROW], f32, tag="ps2")
                nc.tensor.matmul(out=ps[:], lhsT=xt[:], rhs=Wd_sb[:],
                                 start=True, stop=True)
                hsb = h0.tile([P, AROW], f32, tag="hsb2")
                nc.vector.tensor_copy(out=hsb[:], in_=ps[:])
                nc.sync.dma_start(t2[t * P:(t + 1) * P, :], hsb[:])

        # ---------------- Phase 1: edge pass ----------------
        hgp = ctx.enter_context(tc.tile_pool(name="hgp", bufs=2))
        agp = ctx.enter_context(tc.tile_pool(name="agp", bufs=2))
        sp = ctx.enter_context(tc.tile_pool(name="sp", bufs=2))
        msgp = ctx.enter_context(tc.tile_pool(name="msgp", bufs=2))
        smallp = ctx.enter_context(tc.tile_pool(name="smallp", bufs=2))
        idxp = ctx.enter_context(tc.tile_pool(name="idxp", bufs=2))
        psp = ctx.enter_context(tc.tile_pool(name="psp", bufs=4, space="PSUM"))
        stp = ctx.enter_context(tc.tile_pool(name="stp", bufs=2))
        outp = ctx.enter_context(tc.tile_pool(name="outp", bufs=2))

        dl64_sb = consts.tile([P, B_dev], f32)
        nc.sync.dma_start(dl64_sb[:], dl64I[:, :])

        c0 = 0
        for t in range(TPD):
            Kb = int(kb_list[t])
            KbLo = int(kblo_list[t])
            i1 = idxp.tile([P, Kb * 8], i16, tag="i1")
            nc.sync.dma_start(i1[:], idxT1[:, c0 * 8:(c0 + Kb) * 8])
            i2 = idxp.tile([P, Kb * 8], i16, tag="i2")
            nc.sync.dma_start(i2[:], idxT2[:, c0 * 8:(c0 + Kb) * 8])

            hg = hgp.tile([P, Kb * ROW], f32, tag="hg")
            hg3 = hg[:].rearrange("p (k c) -> p k c", c=ROW)
            cs = 0
            while cs < Kb:
                ce = min(cs + CHUNK, Kb)
                if cs < KbLo < ce:
                    ce = KbLo  # split at the lo/hi subset boundary
                tbl = t1lo if cs < KbLo else t1hi
                ni = (ce - cs) * P
                nc.gpsimd.dma_gather(
                    hg3[:, cs:ce, :], tbl[:, :], i1[:, cs * 8:ce * 8],
                    ni, ni, ROW,
                )
                cs = ce
            ag = agp.tile([P, Kb * AROW], f32, tag="ag")
            ag3 = ag[:].rearrange("p (k c) -> p k c", c=AROW)
            for cs in range(0, Kb, CHUNK):
                ce = min(cs + CHUNK, Kb)
                ni = (ce - cs) * P
                nc.gpsimd.dma_gather(
                    ag3[:, cs:ce, :], t2[:, :], i2[:, cs * 8:ce * 8],
                    ni, ni, AROW,
                )

            lg = smallp.tile([P, Kb * 4], f32, tag="lg")
            nc.vector.tensor_tensor(
                out=lg[:].rearrange("p (k c) -> p k c", c=4),
                in0=hg3[:, :, 128:132], in1=ag3[:, :, 0:4], op=OP.add,
            )
            tmp = smallp.tile([P, Kb * 4], f32, tag="tmp")
            nc.vector.tensor_scalar(
                out=tmp[:], in0=lg[:], scalar1=NEG_SLOPE, scalar2=None, op0=OP.mult,
            )
            lk = smallp.tile([P, Kb * 4], f32, tag="lk")
            nc.vector.tensor_tensor(out=lk[:], in0=lg[:], in1=tmp[:], op=OP.max)
            p_all = smallp.tile([P, Kb * 4], f32, tag="p_all")
            nc.scalar.activation(out=p_all[:], in_=lk[:], func=AF.Exp)

            S = sp.tile([P, Kb * GROUP], f32, tag="S")
            iota_b = iota_sb[:].unsqueeze(1).to_broadcast([P, Kb, GROUP])
            dl_b = dl64_sb[:, c0:c0 + Kb].unsqueeze(2).to_broadcast([P, Kb, GROUP])
            nc.vector.tensor_tensor(
                out=S[:].rearrange("p (k d) -> p k d", d=GROUP),
                in0=iota_b, in1=dl_b, op=OP.is_equal,
            )

            msg = msgp.tile([P, Kb * 132], f32, tag="msg")
            msg3 = msg[:].rearrange("p (k c) -> p k c", c=132)
            p3 = p_all[:].rearrange("p (k c) -> p k c", c=4)
            for h in range(4):
                nc.vector.tensor_tensor(
                    out=msg3[:, :, h * 32:(h + 1) * 32],
                    in0=hg3[:, :, h * 32:(h + 1) * 32],
                    in1=p3[:, :, h:h + 1].to_broadcast([P, Kb, 32]),
                    op=OP.mult,
                )
            nc.vector.tensor_copy(out=msg3[:, :, 128:132], in_=p3)

            S3 = S[:].rearrange("p (k d) -> p k d", d=GROUP)
            pgs = [
                psp.tile([GROUP, 132], f32, tag=f"pg{g}", name=f"pg{g}_{t}")
                for g in range(2)
            ]
            # block -> group from [g0s0, g1s0, g0s1, g1s1] run structure
            runs = [(0, int(B4[t, 0, 0])), (1, int(B4[t, 1, 0])),
                    (0, int(B4[t, 0, 1])), (1, int(B4[t, 1, 1]))]
            groups = []
            for g, n in runs:
                groups += [g] * n
            first_of = {g: groups.index(g) for g in (0, 1) if g in groups}
            last_of = {g: len(groups) - 1 - groups[::-1].index(g)
                       for g in (0, 1) if g in groups}
            for c in range(Kb):
                g = groups[c]
                nc.tensor.matmul(
                    out=pgs[g][:], lhsT=S3[:, c, :], rhs=msg3[:, c, :],
                    start=(c == first_of[g]), stop=(c == last_of[g]),
                )

            stage = stp.tile([P, 132], f32, tag="stage")
            for g in (0, 1):
                if g in first_of:
                    nc.vector.tensor_copy(
                        out=stage[g * GROUP:(g + 1) * GROUP, :], in_=pgs[g][:])
                else:
                    nc.vector.memset(stage[g * GROUP:(g + 1) * GROUP, :], 0.0)
            den = smallp.tile([P, 4], f32, tag="den")
            nc.vector.tensor_scalar(
                out=den[:], in0=stage[:, 128:132], scalar1=1e-3, scalar2=1e-16,
                op0=OP.max, op1=OP.add,
            )
            inv = smallp.tile([P, AROW], f32, tag="inv")
            nc.vector.reciprocal(out=inv[:, 0:4], in_=den[:])
            outsb = outp.tile([P, IN_CH], f32, tag="outsb")
            for h in range(4):
                nc.scalar.activation(
                    out=outsb[:, h * 32:(h + 1) * 32],
                    in_=stage[:, h * 32:(h + 1) * 32],
                    func=AF.Copy, scale=inv[:, h:h + 1],
                )
            nc.sync.dma_start(out_d[t * P:(t + 1) * P, :], outsb[:])
            nc.sync.dma_start(invd[t * P:(t + 1) * P, 0:4], inv[:, 0:4])
            nc.sync.dma_start(pdram[:, c0 * 4:(c0 + Kb) * 4], p_all[:])
            c0 += Kb

        # ---------------- Phase 2: alpha = p * invden[dst] ----------------
        p2p = ctx.enter_context(tc.tile_pool(name="p2p", bufs=2))
        for cs in range(0, B_dev, CHUNK):
            ce = min(cs + CHUNK, B_dev)
            K = ce - cs
            psb = p2p.tile([P, K * 4], f32, tag="psb")
            nc.sync.dma_start(psb[:], pdram[:, cs * 4:ce * 4])
            dsb = p2p.tile([P, K * 8], i16, tag="dsb")
            nc.sync.dma_start(dsb[:], idxT2[:, cs * 8:ce * 8])
            ig = p2p.tile([P, K * AROW], f32, tag="ig")
            nc.gpsimd.dma_gather(
                ig[:].rearrange("p (k c) -> p k c", c=AROW), invd[:, :],
                dsb[:], K * P, K * P, AROW,
            )
            asb = p2p.tile([P, K * 4], f32, tag="asb")
            nc.vector.tensor_tensor(
                out=asb[:].rearrange("p (k c) -> p k c", c=4),
                in0=psb[:].rearrange("p (k c) -> p k c", c=4),
                in1=ig[:].rearrange("p (k c) -> p k c", c=AROW)[:, :, 0:4],
                op=OP.mult,
            )
            nc.sync.dma_start(alpha_d[:, cs * 4:ce * 4], asb[:])

    nc.compile()
    return nc


def _time_bass(nc, in_maps, iters=3):
    """Time repeated executions with device-resident inputs (bass2jax path)."""
    import time
    import jax
    import concourse.mybir as mybir
    from concourse import bass2jax
    from concourse.bass2jax import _bass_exec_p, partition_id_tensor
    from jax.sharding import Mesh, PartitionSpec
    from jax.experimental.shard_map import shard_map

    bass2jax.install_neuronx_cc_hook()
    n_cores = len(in_maps)
    partition_name = nc.partition_id_tensor.name if nc.partition_id_tensor else None
    in_names, out_names, out_avals, zero_outs = [], [], [], []
    for alloc in nc.m.functions[0].allocations:
        if not isinstance(alloc, mybir.MemoryLocationSet):
            continue
        name = alloc.memorylocations[0].name
        if alloc.kind == "ExternalInput":
            if name != partition_name:
                in_names.append(name)
        elif alloc.kind == "ExternalOutput":
            out_names.append(name)
            shape = tuple(alloc.tensor_shape)
            dtype = mybir.dt.np(alloc.dtype)
            out_avals.append(jax.core.ShapedArray(shape, dtype))
            zero_outs.append(np.zeros(shape, dtype))
    n_params = len(in_names)
    n_outs = len(out_avals)
    in_names = in_names + out_names
    if partition_name is not None:
        in_names.append(partition_name)
    donate = tuple(range(n_params, n_params + n_outs))

    def _body(*args):
        operands = list(args)
        if partition_name is not None:
            operands.append(partition_id_tensor())
        return tuple(_bass_exec_p.bind(
            *operands, out_avals=tuple(out_avals), in_names=tuple(in_names),
            out_names=tuple(out_names), lowering_input_output_aliases=(),
            sim_require_finite=True, sim_require_nnan=True, nc=nc,
        ))

    devices = jax.devices()[:n_cores]
    mesh = Mesh(np.asarray(devices), ("core",))
    sharding = jax.sharding.NamedSharding(mesh, PartitionSpec("core"))
    sharded = jax.jit(
        shard_map(_body, mesh=mesh,
                  in_specs=(PartitionSpec("core"),) * (n_params + n_outs),
                  out_specs=(PartitionSpec("core"),) * n_outs,
                  check_rep=False),
        donate_argnums=donate, keep_unused=True)
    concat_in = jax.block_until_ready([
        jax.device_put(
            np.concatenate([np.asarray(in_maps[c][nm]) for c in range(n_cores)], 0),
            sharding)
        for nm in in_names[:n_params]])

    def fresh_zeros():
        return jax.block_until_ready([
            jax.device_put(np.zeros((n_cores * z.shape[0], *z.shape[1:]), z.dtype),
                           sharding) for z in zero_outs])

    out_arrs = jax.block_until_ready(sharded(*concat_in, *fresh_zeros()))
    results = [
        {nm: np.asarray(out_arrs[i]).reshape(n_cores, *out_avals[i].shape)[c]
         for i, nm in enumerate(out_names)}
        for c in range(n_cores)]
    best = float("inf")
    for _ in range(iters):
        zs = fresh_zeros()
        t0 = time.perf_counter()
        jax.block_until_ready(sharded(*concat_in, *zs))
        best = min(best, time.perf_counter() - t0)
    return results, best


def kernel(x, edge_index, W, att_src, att_dst, bias):
    global _last_results, _last_exec_s
    x = np.asarray(x, dtype=np.float32)
    edge_index = np.asarray(edge_index)
    W = np.asarray(W, dtype=np.float32)
    att_src = np.asarray(att_src, dtype=np.float32)
    att_dst = np.asarray(att_dst, dtype=np.float32)
    bias = np.asarray(bias, dtype=np.float32)

    prep = _prepare(edge_index)
    B_dev = prep["B_dev"]

    key = (tuple(prep["B4"].reshape(-1).tolist()),)
    if key not in _program_cache:
        _program_cache.clear()
        _program_cache[key] = _build_program(prep["B4"], B_dev)
    nc = _program_cache[key]

    xT = np.zeros((P, NPAD), dtype=np.float32)
    xT[:, :N] = x.T
    As = np.zeros((IN_CH, HEADS), dtype=np.float32)
    Ad = np.zeros((IN_CH, HEADS), dtype=np.float32)
    for h in range(HEADS):
        As[h * OUT_CH:(h + 1) * OUT_CH, h] = att_src[h]
        Ad[h * OUT_CH:(h + 1) * OUT_CH, h] = att_dst[h]
    Wf = np.zeros((P, ROW), dtype=np.float32)
    Wf[:, :IN_CH] = W
    Wf[:, IN_CH:IN_CH + HEADS] = W @ As
    Wd = np.zeros((P, AROW), dtype=np.float32)
    Wd[:, 0:4] = W @ Ad
    iota = np.broadcast_to(
        np.arange(GROUP, dtype=np.float32)[None, :], (P, GROUP)).copy()

    in_maps = []
    for d in range(N_DEV):
        nm = prep["node_map"][d]
        xTp = np.zeros((P, NLOC), dtype=np.float32)
        valid = nm >= 0
        xTp[:, valid] = x.T[:, nm[valid]]
        in_maps.append({
            "xT": xT, "xTp": xTp, "Wf": Wf, "Wd": Wd, "iota64": iota,
            "idxT1": np.ascontiguousarray(prep["idxT1"][d]),
            "idxT2": np.ascontiguousarray(prep["idxT2"][d]),
            "dl64I": np.ascontiguousarray(prep["dl64"][d]),
        })

    iters = int(os.environ.get("GAT_TIME_ITERS", "0"))
    if iters > 0:
        results, sec = _time_bass(nc, in_maps, iters=iters)
        _last_exec_s = sec
    else:
        from concourse.bass_utils import run_bass_kernel_spmd
        res = run_bass_kernel_spmd(nc, in_maps, list(range(N_DEV)))
        results = res.results
        _last_results = res

    out_full = np.zeros((N, IN_CH), dtype=np.float32)
    alpha_full = np.zeros((prep["E2"], HEADS), dtype=np.float32)
    for d in range(N_DEV):
        r = results[d]
        nm = prep["node_map"][d]
        valid = nm >= 0
        out_full[nm[valid]] = r["out"][valid]
        al = r["alpha"].reshape(P, B_dev, 4)
        om = prep["orig"][d]
        ev = om >= 0
        alpha_full[om[ev]] = al[ev]
    out_full += bias[None, :]
    return out_full, alpha_full


# revision 18
# speedup vs baseline: 1.1588x; 1.1588x over previous
"""GAT (graph attention) layer on 8 Trainium2 NeuronCores.

Strategy (edge sharding by destination node):
  - Host: append self-loops, map each dst node to a (device, tile, pos)
    slot (nodes dealt round-robin by degree rank so per-tile edge counts
    balance across devices), pack per-device edge arrays padded to a
    shared block structure so one SPMD program serves all 8 cores.
  - Phase 0 (replicated): h_ext = x @ [W | W@As] via PE with a host-
    pre-transposed x_T; rows [h(128)|a_src(4)|pad] stored as two DRAM
    tables (row id < 32768 / >= 32768) of 768B rows for int16 dma_gather.
    A second small pass computes a_dst for the device's own (permuted)
    nodes into a 256B-row table.
  - Phase 1: per 128-dst-node tile: dma_gather of h_ext rows by src
    (lo/hi subsets in separate block runs), dma_gather of a_dst rows by
    local dst id, p = exp(leaky_relu(a_src+a_dst)), msg = [p*h | p],
    one-hot S via batched is_equal, one PE matmul per 128-edge block
    accumulating weighted sums + softmax denominators into PSUM.  Each
    device owns its dst nodes completely -> no collectives.
  - Phase 2: alpha = p * (1/denom)[dst] via 256B-row dma_gather.
  - Host: un-permute rows, add bias, scatter alpha to original order.
"""

import os
import sys

import numpy as np

sys.path.insert(0, "/opt/trn_rl_repo")

N = 50000
E = 1600000
IN_CH = 128
OUT_CH = 32
HEADS = 4
NEG_SLOPE = 0.2

N_DEV = 8
P = 128
N_TILES = 392
NPAD = N_TILES * P       # 50176
TPD = N_TILES // N_DEV   # 49
NLOC = TPD * P           # 6272
GROUP = 64
LO = 32768               # first LO node ids in table-lo, rest in table-hi
NHI = NPAD - LO          # 17408
ROW = 192                # 768B gathered h_ext row (132 used)
AROW = 64                # 256B a_dst / invden row (4 used)
CHUNK = 8                # gather chunk in blocks (1024 idx, <=3072 desc)

_last_results = None
_last_exec_s = None


def _prepare(edge_index):
    src = np.concatenate([edge_index[0], np.arange(N, dtype=np.int64)]).astype(np.int64)
    dst = np.concatenate([edge_index[1], np.arange(N, dtype=np.int64)]).astype(np.int64)
    E2 = src.shape[0]

    deg = np.bincount(dst, minlength=N)
    order = np.argsort(-deg, kind="stable")
    rank = np.empty(N, dtype=np.int64)
    rank[order] = np.arange(N)
    tile_of = rank % N_TILES
    pos_of = rank // N_TILES
    dev_of = (tile_of % N_DEV).astype(np.int64)
    slot_of = tile_of // N_DEV

    ed_dev = dev_of[dst]
    ed_slot = slot_of[dst]
    ed_pos = pos_of[dst]
    ed_g = ed_pos // GROUP
    ed_sub = (src >= LO).astype(np.int64)

    # cell = (dev, slot, g, sub)
    cell = (((ed_dev * TPD + ed_slot) * 2 + ed_g) * 2 + ed_sub)
    n_cells = N_DEV * TPD * 4
    ordr = np.argsort(cell, kind="stable")
    cell_sorted = cell[ordr]
    counts = np.bincount(cell_sorted, minlength=n_cells).reshape(N_DEV, TPD, 2, 2)

    B4 = -(-counts.max(axis=0) // P)          # [TPD, 2, 2] blocks per (g, sub)
    empty = B4.sum(axis=(1, 2)) == 0
    B4[empty, 0, 0] = 1

    # within-tile block order: [g0s0, g1s0, g0s1, g1s1] (lo run then hi run)
    tile_kb = B4.sum(axis=(1, 2))             # [TPD]
    tile_c0 = np.concatenate([[0], np.cumsum(tile_kb)[:-1]])
    off4 = np.zeros((TPD, 2, 2), dtype=np.int64)
    off4[:, 0, 0] = 0
    off4[:, 1, 0] = B4[:, 0, 0]
    off4[:, 0, 1] = B4[:, 0, 0] + B4[:, 1, 0]
    off4[:, 1, 1] = off4[:, 0, 1] + B4[:, 0, 1]
    B_dev = int(tile_kb.sum())

    cell_start = np.zeros(n_cells + 1, dtype=np.int64)
    np.cumsum(np.bincount(cell_sorted, minlength=n_cells), out=cell_start[1:])
    k = np.arange(E2, dtype=np.int64) - cell_start[cell_sorted]

    e_dev = ed_dev[ordr]
    e_slot = ed_slot[ordr]
    e_g = ed_g[ordr]
    e_sub = ed_sub[ordr]
    c_idx = tile_c0[e_slot] + off4[e_slot, e_g, e_sub] + k // P
    p_idx = k % P

    e_src16 = (src[ordr] - e_sub * LO).astype(np.int16)
    e_dstl = (e_slot * P + ed_pos[ordr]).astype(np.int16)

    # universal wrapped-16 idx layout: [p%16, c*8 + p//16]
    idxT1 = np.zeros((N_DEV, 16, B_dev * 8), dtype=np.int16)
    idxT2 = np.zeros((N_DEV, 16, B_dev * 8), dtype=np.int16)
    col = c_idx * 8 + p_idx // 16
    row = p_idx % 16
    idxT1[e_dev, row, col] = e_src16
    idxT2[e_dev, row, col] = e_dstl

    dl64_arr = np.full((N_DEV, P, B_dev), -1.0, dtype=np.float32)
    orig_arr = np.full((N_DEV, P, B_dev), -1, dtype=np.int64)
    dl64_arr[e_dev, p_idx, c_idx] = (ed_pos[ordr] % GROUP).astype(np.float32)
    orig_arr[e_dev, p_idx, c_idx] = ordr

    node_map = np.full((N_DEV, NLOC), -1, dtype=np.int64)
    node_map[dev_of, slot_of * P + pos_of] = np.arange(N)

    return dict(
        B4=B4, tile_kb=tile_kb, B_dev=B_dev,
        idxT1=np.tile(idxT1, (1, 8, 1)), idxT2=np.tile(idxT2, (1, 8, 1)),
        dl64=dl64_arr, orig=orig_arr, node_map=node_map, E2=E2,
    )


_program_cache = {}


def _build_program(B4, B_dev):
    import concourse.bacc as bacc
    import concourse.bass as bass
    import concourse.mybir as mybir
    import concourse.tile as tile
    from concourse import library_config
    from contextlib import ExitStack

    f32 = mybir.dt.float32
    i16 = mybir.dt.int16
    AF = mybir.ActivationFunctionType
    OP = mybir.AluOpType

    nc = bacc.Bacc("TRN2", target_bir_lowering=False, debug=False)

    xT = nc.dram_tensor("xT", [P, NPAD], f32, kind="ExternalInput")
    xTp = nc.dram_tensor("xTp", [P, NLOC], f32, kind="ExternalInput")
    Wf = nc.dram_tensor("Wf", [P, ROW], f32, kind="ExternalInput")
    Wd = nc.dram_tensor("Wd", [P, AROW], f32, kind="ExternalInput")
    iota = nc.dram_tensor("iota64", [P, GROUP], f32, kind="ExternalInput")
    idxT1 = nc.dram_tensor("idxT1", [P, B_dev * 8], i16, kind="ExternalInput")
    idxT2 = nc.dram_tensor("idxT2", [P, B_dev * 8], i16, kind="ExternalInput")
    dl64I = nc.dram_tensor("dl64I", [P, B_dev], f32, kind="ExternalInput")

    out_d = nc.dram_tensor("out", [NLOC, IN_CH], f32, kind="ExternalOutput")
    alpha_d = nc.dram_tensor("alpha", [P, B_dev * 4], f32, kind="ExternalOutput")

    t1lo = nc.dram_tensor("t1lo", [LO, ROW], f32)
    t1hi = nc.dram_tensor("t1hi", [NHI, ROW], f32)
    t2 = nc.dram_tensor("t2", [NLOC, AROW], f32)
    invd = nc.dram_tensor("invd", [NLOC, AROW], f32)

    kb_list = B4.sum(axis=(1, 2)).astype(int)
    kblo_list = B4[:, :, 0].sum(axis=1).astype(int)

    with tile.TileContext(nc) as tc, ExitStack() as ctx:
        consts = ctx.enter_context(tc.tile_pool(name="consts", bufs=1))
        nc.gpsimd.load_library(library_config.mlp)
        W_sb = consts.tile([P, ROW], f32)
        nc.sync.dma_start(W_sb[:], Wf[:, :])
        Wd_sb = consts.tile([P, AROW], f32)
        nc.sync.dma_start(Wd_sb[:], Wd[:, :])
        iota_sb = consts.tile([P, GROUP], f32)
        nc.sync.dma_start(iota_sb[:], iota[:, :])

        # ---------------- Phase 0: h_ext tables + a_dst table ----------------
        with ExitStack() as ctx0:
            xp = ctx0.enter_context(tc.tile_pool(name="xp", bufs=3))
            h0 = ctx0.enter_context(tc.tile_pool(name="h0", bufs=4))
            ps0 = ctx0.enter_context(tc.tile_pool(name="ps0", bufs=4, space="PSUM"))
            for chb in range(NPAD // 512):
                xt = xp.tile([P, 512], f32, tag="xt")
                nc.sync.dma_start(xt[:], xT[:, chb * 512:(chb + 1) * 512])
                for j in range(4):
                    ti = chb * 4 + j
                    ps = ps0.tile([P, ROW], f32, tag="ps")
                    nc.tensor.matmul(
                        out=ps[:], lhsT=xt[:, j * P:(j + 1) * P], rhs=W_sb[:],
                        start=True, stop=True,
                    )
                    hsb = h0.tile([P, ROW], f32, tag="hsb")
                    if j % 2 == 0:
                        nc.vector.tensor_copy(out=hsb[:], in_=ps[:])
                    else:
                        nc.scalar.copy(out=hsb[:], in_=ps[:])
                    # full 768B rows -> one contiguous 98KB store per tile
                    if ti < LO // P:
                        dest = t1lo[ti * P:(ti + 1) * P, :]
                    else:
                        r0 = ti * P - LO
                        dest = t1hi[r0:r0 + P, :]
                    nc.sync.dma_start(dest, hsb[:])
            # a_dst for this device's own nodes, in local (permuted) order
            for t in range(TPD):
                xt = xp.tile([P, P], f32, tag="xt2")
                nc.sync.dma_start(xt[:], xTp[:, t * P:(t + 1) * P])
                ps = ps0.tile([P, AROW], f32, tag="ps2")
                nc.tensor.matmul(out=ps[:], lhsT=xt[:], rhs=Wd_sb[:],
                                 start=True, stop=True)
                hsb = h0.tile([P, AROW], f32, tag="hsb2")
                nc.vector.tensor_copy(out=hsb[:], in_=ps[:])
                nc.sync.dma_start(t2[t * P:(t + 1) * P, :], hsb[:])

        # ---------------- Phase 1: edge pass ----------------
        hgp = ctx.enter_context(tc.tile_pool(name="hgp", bufs=2))
        agp = ctx.enter_context(tc.tile_pool(name="agp", bufs=2))
        sp = ctx.enter_context(tc.tile_pool(name="sp", bufs=2))
        msgp = ctx.enter_context(tc.tile_pool(name="msgp", bufs=2))
        smallp = ctx.enter_context(tc.tile_pool(name="smallp", bufs=2))
        idxp = ctx.enter_context(tc.tile_pool(name="idxp", bufs=2))
        psp = ctx.enter_context(tc.tile_pool(name="psp", bufs=4, space="PSUM"))
        stp = ctx.enter_context(tc.tile_pool(name="stp", bufs=2))
        outp = ctx.enter_context(tc.tile_pool(name="outp", bufs=2))
        p2p = ctx.enter_context(tc.tile_pool(name="p2p", bufs=2))

        dl64_sb = consts.tile([P, B_dev], f32)
        nc.sync.dma_start(dl64_sb[:], dl64I[:, :])

        c0 = 0
        for t in range(TPD):
            Kb = int(kb_list[t])
            KbLo = int(kblo_list[t])
            i1 = idxp.tile([P, Kb * 8], i16, tag="i1")
            nc.sync.dma_start(i1[:], idxT1[:, c0 * 8:(c0 + Kb) * 8])
            i2 = idxp.tile([P, Kb * 8], i16, tag="i2")
            nc.sync.dma_start(i2[:], idxT2[:, c0 * 8:(c0 + Kb) * 8])

            hg = hgp.tile([P, Kb * ROW], f32, tag="hg")
            hg3 = hg[:].rearrange("p (k c) -> p k c", c=ROW)
            cs = 0
            while cs < Kb:
                ce = min(cs + CHUNK, Kb)
                if cs < KbLo < ce:
                    ce = KbLo  # split at the lo/hi subset boundary
                tbl = t1lo if cs < KbLo else t1hi
                ni = (ce - cs) * P
                nc.gpsimd.dma_gather(
                    hg3[:, cs:ce, :], tbl[:, :], i1[:, cs * 8:ce * 8],
                    ni, ni, ROW,
                )
                cs = ce
            ag = agp.tile([P, Kb * AROW], f32, tag="ag")
            ag3 = ag[:].rearrange("p (k c) -> p k c", c=AROW)
            for cs in range(0, Kb, CHUNK):
                ce = min(cs + CHUNK, Kb)
                ni = (ce - cs) * P
                nc.gpsimd.dma_gather(
                    ag3[:, cs:ce, :], t2[:, :], i2[:, cs * 8:ce * 8],
                    ni, ni, AROW,
                )

            lg = smallp.tile([P, Kb * 4], f32, tag="lg")
            nc.vector.tensor_tensor(
                out=lg[:].rearrange("p (k c) -> p k c", c=4),
                in0=hg3[:, :, 128:132], in1=ag3[:, :, 0:4], op=OP.add,
            )
            tmp = smallp.tile([P, Kb * 4], f32, tag="tmp")
            nc.vector.tensor_scalar(
                out=tmp[:], in0=lg[:], scalar1=NEG_SLOPE, scalar2=None, op0=OP.mult,
            )
            lk = smallp.tile([P, Kb * 4], f32, tag="lk")
            nc.vector.tensor_tensor(out=lk[:], in0=lg[:], in1=tmp[:], op=OP.max)
            p_all = smallp.tile([P, Kb * 4], f32, tag="p_all")
            nc.scalar.activation(out=p_all[:], in_=lk[:], func=AF.Exp)

            S = sp.tile([P, Kb * GROUP], f32, tag="S")
            iota_b = iota_sb[:].unsqueeze(1).to_broadcast([P, Kb, GROUP])
            dl_b = dl64_sb[:, c0:c0 + Kb].unsqueeze(2).to_broadcast([P, Kb, GROUP])
            nc.vector.tensor_tensor(
                out=S[:].rearrange("p (k d) -> p k d", d=GROUP),
                in0=iota_b, in1=dl_b, op=OP.is_equal,
            )

            msg = msgp.tile([P, Kb * 132], f32, tag="msg")
            msg3 = msg[:].rearrange("p (k c) -> p k c", c=132)
            p3 = p_all[:].rearrange("p (k c) -> p k c", c=4)
            for h in range(4):
                nc.vector.tensor_tensor(
                    out=msg3[:, :, h * 32:(h + 1) * 32],
                    in0=hg3[:, :, h * 32:(h + 1) * 32],
                    in1=p3[:, :, h:h + 1].to_broadcast([P, Kb, 32]),
                    op=OP.mult,
                )
            nc.vector.tensor_copy(out=msg3[:, :, 128:132], in_=p3)

            S3 = S[:].rearrange("p (k d) -> p k d", d=GROUP)
            pgs = [
                psp.tile([GROUP, 132], f32, tag=f"pg{g}", name=f"pg{g}_{t}")
                for g in range(2)
            ]
            # block -> group from [g0s0, g1s0, g0s1, g1s1] run structure
            runs = [(0, int(B4[t, 0, 0])), (1, int(B4[t, 1, 0])),
                    (0, int(B4[t, 0, 1])), (1, int(B4[t, 1, 1]))]
            groups = []
            for g, n in runs:
                groups += [g] * n
            first_of = {g: groups.index(g) for g in (0, 1) if g in groups}
            last_of = {g: len(groups) - 1 - groups[::-1].index(g)
                       for g in (0, 1) if g in groups}
            for c in range(Kb):
                g = groups[c]
                nc.tensor.matmul(
                    out=pgs[g][:], lhsT=S3[:, c, :], rhs=msg3[:, c, :],
                    start=(c == first_of[g]), stop=(c == last_of[g]),
                )

            stage = stp.tile([P, 132], f32, tag="stage")
            for g in (0, 1):
                if g in first_of:
                    nc.vector.tensor_copy(
                        out=stage[g * GROUP:(g + 1) * GROUP, :], in_=pgs[g][:])
                else:
                    nc.vector.memset(stage[g * GROUP:(g + 1) * GROUP, :], 0.0)
            den = smallp.tile([P, 4], f32, tag="den")
            nc.vector.tensor_scalar(
                out=den[:], in0=stage[:, 128:132], scalar1=1e-3, scalar2=1e-16,
                op0=OP.max, op1=OP.add,
            )
            inv = smallp.tile([P, AROW], f32, tag="inv")
            nc.vector.reciprocal(out=inv[:, 0:4], in_=den[:])
            outsb = outp.tile([P, IN_CH], f32, tag="outsb")
            for h in range(4):
                nc.scalar.activation(
                    out=outsb[:, h * 32:(h + 1) * 32],
                    in_=stage[:, h * 32:(h + 1) * 32],
                    func=AF.Copy, scale=inv[:, h:h + 1],
                )
            nc.sync.dma_start(out_d[t * P:(t + 1) * P, :], outsb[:])
            nc.sync.dma_start(invd[t * P:(t + 1) * P, 0:4], inv[:, 0:4])
            # fused alpha normalization: gather the just-written invd rows
            # (this tile's edges only reference this tile's rows) and scale
            # the still-SBUF-resident p
            for cs2 in range(0, Kb, CHUNK):
                ce2 = min(cs2 + CHUNK, Kb)
                Kc = ce2 - cs2
                ig = p2p.tile([P, Kc * AROW], f32, tag="ig",
                              name=f"ig_{t}_{cs2}")
                nc.gpsimd.dma_gather(
                    ig[:].rearrange("p (k c) -> p k c", c=AROW), invd[:, :],
                    i2[:, cs2 * 8:ce2 * 8], Kc * P, Kc * P, AROW,
                )
                asb = p2p.tile([P, Kc * 4], f32, tag="asb",
                               name=f"asb_{t}_{cs2}")
                nc.vector.tensor_tensor(
                    out=asb[:].rearrange("p (k c) -> p k c", c=4),
                    in0=p3[:, cs2:ce2, :],
                    in1=ig[:].rearrange("p (k c) -> p k c", c=AROW)[:, :, 0:4],
                    op=OP.mult,
                )
                nc.sync.dma_start(
                    alpha_d[:, (c0 + cs2) * 4:(c0 + ce2) * 4], asb[:])
            c0 += Kb

    nc.compile()
    return nc


def _time_bass(nc, in_maps, iters=3):
    """Time repeated executions with device-resident inputs (bass2jax path)."""
    import time
    import jax
    import concourse.mybir as mybir
    from concourse import bass2jax
    from concourse.bass2jax import _bass_exec_p, partition_id_tensor
    from jax.sharding import Mesh, PartitionSpec
    from jax.experimental.shard_map import shard_map

    bass2jax.install_neuronx_cc_hook()
    n_cores = len(in_maps)
    partition_name = nc.partition_id_tensor.name if nc.partition_id_tensor else None
    in_names, out_names, out_avals, zero_outs = [], [], [], []
    for alloc in nc.m.functions[0].allocations:
        if not isinstance(alloc, mybir.MemoryLocationSet):
            continue
        name = alloc.memorylocations[0].name
        if alloc.kind == "ExternalInput":
            if name != partition_name:
                in_names.append(name)
        elif alloc.kind == "ExternalOutput":
            out_names.append(name)
            shape = tuple(alloc.tensor_shape)
            dtype = mybir.dt.np(alloc.dtype)
            out_avals.append(jax.core.ShapedArray(shape, dtype))
            zero_outs.append(np.zeros(shape, dtype))
    n_params = len(in_names)
    n_outs = len(out_avals)
    in_names = in_names + out_names
    if partition_name is not None:
        in_names.append(partition_name)
    donate = tuple(range(n_params, n_params + n_outs))

    def _body(*args):
        operands = list(args)
        if partition_name is not None:
            operands.append(partition_id_tensor())
        return tuple(_bass_exec_p.bind(
            *operands, out_avals=tuple(out_avals), in_names=tuple(in_names),
            out_names=tuple(out_names), lowering_input_output_aliases=(),
            sim_require_finite=True, sim_require_nnan=True, nc=nc,
        ))

    devices = jax.devices()[:n_cores]
    mesh = Mesh(np.asarray(devices), ("core",))
    sharding = jax.sharding.NamedSharding(mesh, PartitionSpec("core"))
    sharded = jax.jit(
        shard_map(_body, mesh=mesh,
                  in_specs=(PartitionSpec("core"),) * (n_params + n_outs),
                  out_specs=(PartitionSpec("core"),) * n_outs,
                  check_rep=False),
        donate_argnums=donate, keep_unused=True)
    concat_in = jax.block_until_ready([
        jax.device_put(
            np.concatenate([np.asarray(in_maps[c][nm]) for c in range(n_cores)], 0),
            sharding)
        for nm in in_names[:n_params]])

    def fresh_zeros():
        return jax.block_until_ready([
            jax.device_put(np.zeros((n_cores * z.shape[0], *z.shape[1:]), z.dtype),
                           sharding) for z in zero_outs])

    out_arrs = jax.block_until_ready(sharded(*concat_in, *fresh_zeros()))
    results = [
        {nm: np.asarray(out_arrs[i]).reshape(n_cores, *out_avals[i].shape)[c]
         for i, nm in enumerate(out_names)}
        for c in range(n_cores)]
    best = float("inf")
    for _ in range(iters):
        zs = fresh_zeros()
        t0 = time.perf_counter()
        jax.block_until_ready(sharded(*concat_in, *zs))
        best = min(best, time.perf_counter() - t0)
    return results, best


def kernel(x, edge_index, W, att_src, att_dst, bias):
    global _last_results, _last_exec_s
    x = np.asarray(x, dtype=np.float32)
    edge_index = np.asarray(edge_index)
    W = np.asarray(W, dtype=np.float32)
    att_src = np.asarray(att_src, dtype=np.float32)
    att_dst = np.asarray(att_dst, dtype=np.float32)
    bias = np.asarray(bias, dtype=np.float32)

    prep = _prepare(edge_index)
    B_dev = prep["B_dev"]

    key = (tuple(prep["B4"].reshape(-1).tolist()),)
    if key not in _program_cache:
        _program_cache.clear()
        _program_cache[key] = _build_program(prep["B4"], B_dev)
    nc = _program_cache[key]

    xT = np.zeros((P, NPAD), dtype=np.float32)
    xT[:, :N] = x.T
    As = np.zeros((IN_CH, HEADS), dtype=np.float32)
    Ad = np.zeros((IN_CH, HEADS), dtype=np.float32)
    for h in range(HEADS):
        As[h * OUT_CH:(h + 1) * OUT_CH, h] = att_src[h]
        Ad[h * OUT_CH:(h + 1) * OUT_CH, h] = att_dst[h]
    Wf = np.zeros((P, ROW), dtype=np.float32)
    Wf[:, :IN_CH] = W
    Wf[:, IN_CH:IN_CH + HEADS] = W @ As
    Wd = np.zeros((P, AROW), dtype=np.float32)
    Wd[:, 0:4] = W @ Ad
    iota = np.broadcast_to(
        np.arange(GROUP, dtype=np.float32)[None, :], (P, GROUP)).copy()

    in_maps = []
    for d in range(N_DEV):
        nm = prep["node_map"][d]
        xTp = np.zeros((P, NLOC), dtype=np.float32)
        valid = nm >= 0
        xTp[:, valid] = x.T[:, nm[valid]]
        in_maps.append({
            "xT": xT, "xTp": xTp, "Wf": Wf, "Wd": Wd, "iota64": iota,
            "idxT1": np.ascontiguousarray(prep["idxT1"][d]),
            "idxT2": np.ascontiguousarray(prep["idxT2"][d]),
            "dl64I": np.ascontiguousarray(prep["dl64"][d]),
        })

    iters = int(os.environ.get("GAT_TIME_ITERS", "0"))
    if iters > 0:
        results, sec = _time_bass(nc, in_maps, iters=iters)
        _last_exec_s = sec
    else:
        from concourse.bass_utils import run_bass_kernel_spmd
        res = run_bass_kernel_spmd(nc, in_maps, list(range(N_DEV)))
        results = res.results
        _last_results = res

    out_full = np.zeros((N, IN_CH), dtype=np.float32)
    alpha_full = np.zeros((prep["E2"], HEADS), dtype=np.float32)
    for d in range(N_DEV):
        r = results[d]
        nm = prep["node_map"][d]
        valid = nm >= 0
        out_full[nm[valid]] = r["out"][valid]
        al = r["alpha"].reshape(P, B_dev, 4)
        om = prep["orig"][d]
        ev = om >= 0
        alpha_full[om[ev]] = al[ev]
    out_full += bias[None, :]
    return out_full, alpha_full
